# revision 22
# baseline (speedup 1.0000x reference)
"""Trainium2 Bass kernel for BidirectionalCrossModalCausalAttention.

Shapes (hardcoded): B=64, S=4, C=1280, HID=256, H=W=32.
Sharding: data-parallel over batch: 8 samples per NeuronCore, weights replicated.

Per sample (x = visual_features[b] as (C, HW)):
  [k; v] = [Wk; Wv] @ x + [bk; bv]    -- one stacked fp32r GEMM, weights stationary
  scores = (q.k) / (max(|q|,eps) * max(|k|,eps));  attn = softmax over hw
  pooled = sum_hw v*attn;  visual_vector = pooled @ (Wp.T/1024) + bp
  gap = mean_hw x -> MLP -> softmax -> sensor_weights;  recal = sensor * sw
Softmax/attn/pooling run in half-batches of 4 samples so the first half
overlaps the second half's GEMMs.
"""
import numpy as np
from contextlib import ExitStack

import concourse.bass as bass
import concourse.tile as tile
from concourse import bacc, mybir
from concourse.bass_utils import run_bass_kernel_spmd

B, S, C, HID, H, W = 64, 4, 1280, 256, 32, 32
HW = H * W
NCORES = 8
BS = B // NCORES          # 8 samples per core
HB = BS // 2              # half-batch of 4
NK = C // 128             # 10 contraction tiles
NM = (2 * HID) // 128     # 4 output row-tiles (2 k + 2 v)
EPS = 1e-8
F32 = mybir.dt.float32
F32R = mybir.dt.float32r
AX = mybir.AxisListType.X
ALU = mybir.AluOpType
AF = mybir.ActivationFunctionType

_CACHE = {}


def _build():
    nc = bacc.Bacc("TRN2", target_bir_lowering=False, debug=False, num_devices=NCORES)

    dram = lambda nm, sh, kind: nc.dram_tensor(nm, sh, F32, kind=kind).ap()
    d_x = dram("x", [BS, C, HW], "ExternalInput")
    d_sensor = dram("sensor", [BS, S], "ExternalInput")
    d_sensorT = dram("sensorT", [S, BS], "ExternalInput")
    d_wallT = dram("wallT", [C, 2 * HID], "ExternalInput")
    d_wq = dram("wq", [S, HID], "ExternalInput")
    d_w1s = dram("w1s", [C, HID], "ExternalInput")
    d_w2 = dram("w2", [HID, S], "ExternalInput")
    d_wpTs = dram("wpTs", [HID, C], "ExternalInput")
    d_bk = dram("bk", [HID, 1], "ExternalInput")
    d_bv = dram("bv", [HID, 1], "ExternalInput")
    d_b1 = dram("b1", [HID, 1], "ExternalInput")
    d_bq_col = dram("bq_col", [HID, 1], "ExternalInput")
    d_bq_row = dram("bq_row", [1, HID], "ExternalInput")
    d_bp_row = dram("bp_row", [1, C], "ExternalInput")
    d_b2_row = dram("b2_row", [1, S], "ExternalInput")
    d_ones_col = dram("ones_col", [1, 128], "ExternalInput")
    d_ones_row = dram("ones_row", [1, BS], "ExternalInput")
    d_ones_ck = dram("ones_ck", [128, 1], "ExternalInput")
    d_vtmp = nc.dram_tensor("vtmp", [BS, 2, 128, HW], F32).ap()
    d_attn = dram("attn", [BS, HW], "ExternalOutput")
    d_vv = dram("vv", [BS, C], "ExternalOutput")
    d_sw = dram("sw", [BS, S], "ExternalOutput")
    d_recal = dram("recal", [BS, S], "ExternalOutput")

    with tile.TileContext(nc) as tc, ExitStack() as ctx:
        P = lambda **kw: ctx.enter_context(tc.tile_pool(**kw))
        wpool = P(name="w", bufs=1)
        xpool = P(name="x", bufs=15)
        kpool = P(name="k", bufs=2)
        vpool = P(name="v", bufs=5)
        spool = P(name="s", bufs=1)
        tpool = P(name="t", bufs=3)
        ps_main = P(name="pm", bufs=2, space="PSUM")    # 4 banks
        ps_small = P(name="psm", bufs=2, space="PSUM")  # 2 banks
        ps_bc = P(name="pbc", bufs=1, space="PSUM")     # 2 banks

        mm = nc.tensor.matmul
        act = nc.scalar.activation
        dve = nc.vector

        # ---- early weights (needed for main loop) ----
        wall = []
        for k in range(NK):
            t = wpool.tile([128, 2 * HID], F32R, name=f"wall_{k}")
            nc.sync.dma_start(out=t[:], in_=d_wallT[k * 128:(k + 1) * 128, :].bitcast(F32R))
            wall.append(t)
        wq_t = wpool.tile([S, HID], F32R, name="wq_t")
        nc.sync.dma_start(out=wq_t[:], in_=d_wq.bitcast(F32R))
        sensorT_t = wpool.tile([S, BS], F32R, name="sensorT_t")
        nc.sync.dma_start(out=sensorT_t[:], in_=d_sensorT.bitcast(F32R))
        sensor_t = wpool.tile([BS, S], F32, name="sensor_t")
        nc.sync.dma_start(out=sensor_t[:], in_=d_sensor)
        bq_row = wpool.tile([1, HID], F32R, name="bq_row")
        nc.sync.dma_start(out=bq_row[:], in_=d_bq_row.bitcast(F32R))
        bk_c, bv_c, b1_c, bq_c = [], [], [], []
        for j in range(2):
            for lst, src, nm in ((bk_c, d_bk, "bk"), (bv_c, d_bv, "bv"),
                                 (b1_c, d_b1, "b1"), (bq_c, d_bq_col, "bq")):
                t = wpool.tile([128, 1], F32, name=f"{nm}_c{j}")
                nc.sync.dma_start(out=t[:], in_=src[j * 128:(j + 1) * 128, :])
                lst.append(t)
        ones_col = wpool.tile([1, 128], F32R, name="ones_col")
        nc.sync.dma_start(out=ones_col[:], in_=d_ones_col.bitcast(F32R))
        ones_row = wpool.tile([1, BS], F32R, name="ones_row")
        nc.sync.dma_start(out=ones_row[:], in_=d_ones_row.bitcast(F32R))
        ones_ck = wpool.tile([128, 1], F32R, name="ones_ck")
        nc.sync.dma_start(out=ones_ck[:], in_=d_ones_ck.bitcast(F32R))

        # ---- q path: rq per half-batch + full qT ----
        rq_h, qT_r = [], []
        for h in range(2):
            q_ps = ps_small.tile([HB, HID], F32, tag="small", name=f"q_ps{h}")
            mm(out=q_ps[:], lhsT=sensorT_t[:, h * HB:(h + 1) * HB], rhs=wq_t[:],
               start=True, stop=False)
            mm(out=q_ps[:], lhsT=ones_row[:, 0:HB], rhs=bq_row[:], start=False, stop=True)
            q2s = tpool.tile([HB, HID], F32, tag="tq", name=f"q2s{h}")
            q2sum = spool.tile([HB, 1], F32, name=f"q2sum{h}")
            act(q2s[:], q_ps[:], AF.Square, accum_out=q2sum[:])
            qn = spool.tile([HB, 1], F32, name=f"qn{h}")
            act(qn[:], q2sum[:], AF.Sqrt)
            qnm = spool.tile([HB, 1], F32, name=f"qnm{h}")
            dve.tensor_scalar_max(qnm[:], qn[:], EPS)
            rq = spool.tile([HB, 1], F32, name=f"rq{h}")
            dve.reciprocal(rq[:], qnm[:])
            rq_h.append(rq)
        for j in range(2):
            p = ps_small.tile([128, BS], F32, tag="small", name=f"qT_ps{j}")
            mm(out=p[:], lhsT=wq_t[:, j * 128:(j + 1) * 128], rhs=sensorT_t[:],
               start=True, stop=True)
            t = spool.tile([128, BS], F32R, name=f"qT_r{j}")
            act(t[:], p[:], AF.Identity, bias=bq_c[j][:])
            qT_r.append(t)

        gapT = [spool.tile([128, BS], F32, name=f"gapT_{k}") for k in range(NK)]
        pooledT = [spool.tile([128, BS], F32, name=f"pooledT_{j}") for j in range(2)]
        qk_h = [spool.tile([HB, HW], F32, name=f"qk_h{h}") for h in range(2)]
        n2_h = [spool.tile([HB, HW], F32, name=f"n2_h{h}") for h in range(2)]

        def attn_pool_phase(h):
            """softmax + attn out + pooled for samples h*HB..h*HB+HB-1."""
            rows = range(h * HB, (h + 1) * HB)
            knorm = tpool.tile([HB, HW], F32, tag="tb", name=f"knorm{h}")
            act(knorm[:], n2_h[h][:], AF.Sqrt)
            knm = tpool.tile([HB, HW], F32, tag="tb", name=f"knm{h}")
            dve.tensor_scalar_max(knm[:], knorm[:], EPS)
            rk = tpool.tile([HB, HW], F32, tag="tb", name=f"rk{h}")
            dve.reciprocal(rk[:], knm[:])
            scores = tpool.tile([HB, HW], F32, tag="tb", name=f"scores{h}")
            dve.tensor_tensor(scores[:], qk_h[h][:], rk[:], ALU.mult)
            maxs = spool.tile([HB, 1], F32, name=f"maxs{h}")
            dve.tensor_reduce(maxs[:], scores[:], AX, ALU.max)
            nrq = spool.tile([HB, 1], F32, name=f"nrq{h}")
            dve.tensor_scalar_mul(nrq[:], rq_h[h][:], -1.0)
            bias2 = spool.tile([HB, 1], F32, name=f"bias2{h}")
            dve.tensor_tensor(bias2[:], maxs[:], nrq[:], ALU.mult)
            e_t = tpool.tile([HB, HW], F32, tag="tb", name=f"e_t{h}")
            zsum = spool.tile([HB, 1], F32, name=f"zsum{h}")
            act(e_t[:], scores[:], AF.Exp, bias=bias2[:], scale=rq_h[h][:],
                accum_out=zsum[:])
            rz = spool.tile([HB, 1], F32, name=f"rz{h}")
            dve.reciprocal(rz[:], zsum[:])
            attn_f = tpool.tile([HB, HW], F32, tag="tb", name=f"attn_f{h}")
            act(attn_f[:], e_t[:], AF.Copy, scale=rz[:])
            nc.sync.dma_start(out=d_attn[h * HB:(h + 1) * HB, :], in_=attn_f[:])
            for i, s in enumerate(rows):
                arow = tpool.tile([1, HW], F32R, tag="arow", name=f"arow_{s}", bufs=2)
                nc.sync.dma_start(out=arow[:], in_=attn_f[i:i + 1, :].bitcast(F32R))
                bc = ps_bc.tile([128, HW], F32, tag="bc", name=f"bc_{s}")
                for n in range(2):
                    mm(out=bc[:, n * 512:(n + 1) * 512], lhsT=ones_col[:],
                       rhs=arow[0:1, n * 512:(n + 1) * 512], start=True, stop=True)
                for j in range(2):
                    vt = vpool.tile([128, HW], F32, name=f"vld_{s}_{j}", tag="vsb")
                    nc.gpsimd.dma_start(out=vt[:], in_=d_vtmp[s, j])
                    scr = tpool.tile([128, HW], F32, tag="scr", name=f"scr_{s}_{j}", bufs=2)
                    dve.tensor_tensor(scr[:], vt[:], bc[:], ALU.mult)
                    dve.tensor_reduce(pooledT[j][:, s:s + 1], scr[:], AX, ALU.add)

        # ---- main per-sample loop ----
        for s in range(BS):
            h, i = s // HB, s % HB
            xt = [xpool.tile([128, HW], F32R, name=f"xt_{s}_{k}", tag="xt")
                  for k in range(NK)]
            for k in range(NK):
                eng = nc.gpsimd if k % 2 == 0 else nc.sync
                eng.dma_start(out=xt[k][:], in_=d_x[s, k * 128:(k + 1) * 128, :].bitcast(F32R))
            ks, k2s = [], []
            for m in range(NM):
                acc = ps_main.tile([128, HW], F32, tag="main", name=f"acc_{s}_{m}")
                for k in range(NK):
                    for n in range(2):
                        mm(out=acc[:, n * 512:(n + 1) * 512],
                           lhsT=wall[k][:, m * 128:(m + 1) * 128],
                           rhs=xt[k][:, n * 512:(n + 1) * 512],
                           start=(k == 0), stop=(k == NK - 1))
                if m < 2:
                    t1 = kpool.tile([128, HW], F32R, name=f"ksb_{s}_{m}", tag="ksb")
                    act(t1[:], acc[:], AF.Identity, bias=bk_c[m][:])
                    t2 = kpool.tile([128, HW], F32R, name=f"k2sb_{s}_{m}", tag="k2sb")
                    act(t2[:], acc[:], AF.Square, bias=bk_c[m][:])
                    ks.append(t1)
                    k2s.append(t2)
                else:
                    j = m - 2
                    t = vpool.tile([128, HW], F32, name=f"vsb_{s}_{j}", tag="vsb")
                    act(t[:], acc[:], AF.Identity, bias=bv_c[j][:])
                    nc.gpsimd.dma_start(out=d_vtmp[s, j], in_=t[:])
            for qi, (dst, rhs_t) in enumerate(((qk_h[h], ks), (n2_h[h], k2s))):
                row = tpool.tile([1, HW], F32, tag="row", name=f"row_{s}_{qi}", bufs=2)
                for n in range(2):
                    p = ps_small.tile([1, 512], F32, tag="small", name=f"rps_{s}_{n}_{qi}")
                    for j in range(2):
                        lhs = qT_r[j][:, s:s + 1] if qi == 0 else ones_ck[:]
                        mm(out=p[:], lhsT=lhs, rhs=rhs_t[j][:, n * 512:(n + 1) * 512],
                           start=(j == 0), stop=(j == 1))
                    act(row[0:1, n * 512:(n + 1) * 512], p[:], AF.Copy)
                nc.sync.dma_start(out=dst[i:i + 1, :], in_=row[:])
            for k in range(NK):
                dve.tensor_reduce(gapT[k][:, s:s + 1], xt[k][:].bitcast(F32), AX, ALU.add)
            if i == HB - 1:
                attn_pool_phase(h)

        # ---- late weights ----
        w1s, wpTs, w2t = [], [], []
        for k in range(NK):
            t = wpool.tile([128, HID], F32R, name=f"w1s_{k}")
            nc.sync.dma_start(out=t[:], in_=d_w1s[k * 128:(k + 1) * 128, :].bitcast(F32R))
            w1s.append(t)
        for j in range(2):
            t = wpool.tile([128, C], F32R, name=f"wpTs_{j}")
            nc.sync.dma_start(out=t[:], in_=d_wpTs[j * 128:(j + 1) * 128, :].bitcast(F32R))
            wpTs.append(t)
        for j in range(2):
            t = wpool.tile([128, S], F32R, name=f"w2t_{j}")
            nc.sync.dma_start(out=t[:], in_=d_w2[j * 128:(j + 1) * 128, :].bitcast(F32R))
            w2t.append(t)
        bp_row = wpool.tile([1, C], F32R, name="bp_row")
        nc.sync.dma_start(out=bp_row[:], in_=d_bp_row.bitcast(F32R))
        b2_row = wpool.tile([1, S], F32R, name="b2_row")
        nc.sync.dma_start(out=b2_row[:], in_=d_b2_row.bitcast(F32R))

        # ---- visual_vector = pooled @ (Wp.T/1024) + bp ----
        pooledT_r = []
        for j in range(2):
            t = spool.tile([128, BS], F32R, name=f"pooledT_r{j}")
            act(t[:], pooledT[j][:], AF.Copy)
            pooledT_r.append(t)
        vv_sb = spool.tile([BS, C], F32)
        for nchunk, n0 in ((512, 0), (512, 512), (256, 1024)):
            p = ps_small.tile([BS, 512], F32, tag="small", name=f"vv_ps_{n0}")
            for j in range(2):
                mm(out=p[:, :nchunk], lhsT=pooledT_r[j][:], rhs=wpTs[j][:, n0:n0 + nchunk],
                   start=(j == 0), stop=False)
            mm(out=p[:, :nchunk], lhsT=ones_row[:], rhs=bp_row[:, n0:n0 + nchunk],
               start=False, stop=True)
            act(vv_sb[:, n0:n0 + nchunk], p[:, :nchunk], AF.Copy)
        nc.sync.dma_start(out=d_vv[:, :], in_=vv_sb[:])

        # ---- gap MLP -> sensor weights ----
        gapT_r = []
        for k in range(NK):
            t = spool.tile([128, BS], F32R, name=f"gapT_r{k}")
            act(t[:], gapT[k][:], AF.Copy)
            gapT_r.append(t)
        hiddenT_r = []
        for j in range(2):
            p = ps_small.tile([128, BS], F32, tag="small", name=f"hid_ps{j}")
            for k in range(NK):
                mm(out=p[:], lhsT=w1s[k][:, j * 128:(j + 1) * 128], rhs=gapT_r[k][:],
                   start=(k == 0), stop=(k == NK - 1))
            t = spool.tile([128, BS], F32R, name=f"hiddenT_r{j}")
            act(t[:], p[:], AF.Relu, bias=b1_c[j][:])
            hiddenT_r.append(t)
        lg_ps = ps_small.tile([BS, S], F32, tag="small")
        for j in range(2):
            mm(out=lg_ps[:], lhsT=hiddenT_r[j][:], rhs=w2t[j][:], start=(j == 0), stop=False)
        mm(out=lg_ps[:], lhsT=ones_row[:], rhs=b2_row[:], start=False, stop=True)
        lmax = spool.tile([BS, 1], F32)
        dve.tensor_reduce(lmax[:], lg_ps[:], AX, ALU.max)
        nlmax = spool.tile([BS, 1], F32)
        dve.tensor_scalar_mul(nlmax[:], lmax[:], -1.0)
        le_t = spool.tile([BS, S], F32)
        lz = spool.tile([BS, 1], F32)
        act(le_t[:], lg_ps[:], AF.Exp, bias=nlmax[:], accum_out=lz[:])
        rlz = spool.tile([BS, 1], F32)
        dve.reciprocal(rlz[:], lz[:])
        sw_sb = spool.tile([BS, S], F32)
        act(sw_sb[:], le_t[:], AF.Copy, scale=rlz[:])
        nc.sync.dma_start(out=d_sw[:, :], in_=sw_sb[:])
        recal_sb = spool.tile([BS, S], F32)
        dve.tensor_tensor(recal_sb[:], sensor_t[:], sw_sb[:], ALU.mult)
        nc.sync.dma_start(out=d_recal[:, :], in_=recal_sb[:])

    nc.compile()
    return nc


def _prep_inputs(inputs):
    f = lambda a: np.ascontiguousarray(np.asarray(a, dtype=np.float32))
    sensor = f(inputs["sensor_features"])
    x = f(inputs["visual_features"]).reshape(B, C, HW)
    Wk, Wv = f(inputs["Wk"]), f(inputs["Wv"])
    wallT = np.ascontiguousarray(np.concatenate([Wk.T, Wv.T], axis=1))  # (C, 2*HID)
    shared = {
        "wallT": wallT,
        "wq": f(inputs["Wq"]),
        "w1s": np.ascontiguousarray(f(inputs["W1"]) / HW),
        "w2": f(inputs["W2"]),
        "wpTs": np.ascontiguousarray(f(inputs["Wp"]).T / HW),
        "bk": f(inputs["bk"]).reshape(HID, 1),
        "bv": f(inputs["bv"]).reshape(HID, 1),
        "b1": f(inputs["b1"]).reshape(HID, 1),
        "bq_col": f(inputs["bq"]).reshape(HID, 1),
        "bq_row": f(inputs["bq"]).reshape(1, HID),
        "bp_row": f(inputs["bp"]).reshape(1, C),
        "b2_row": f(inputs["b2"]).reshape(1, S),
        "ones_col": np.ones((1, 128), np.float32),
        "ones_row": np.ones((1, BS), np.float32),
        "ones_ck": np.ones((128, 1), np.float32),
    }
    in_maps = []
    for i in range(NCORES):
        sl = slice(i * BS, (i + 1) * BS)
        m = dict(shared)
        m["x"] = np.ascontiguousarray(x[sl])
        m["sensor"] = np.ascontiguousarray(sensor[sl])
        m["sensorT"] = np.ascontiguousarray(sensor[sl].T)
        in_maps.append(m)
    return in_maps


def kernel(**inputs):
    if "nc" not in _CACHE:
        _CACHE["nc"] = _build()
    nc = _CACHE["nc"]
    in_maps = _prep_inputs(inputs)
    res = run_bass_kernel_spmd(nc, in_maps, list(range(NCORES))).results
    vv = np.concatenate([r["vv"] for r in res], axis=0)
    recal = np.concatenate([r["recal"] for r in res], axis=0)
    attn = np.concatenate([r["attn"] for r in res], axis=0).reshape(B, 1, H, W)
    sw = np.concatenate([r["sw"] for r in res], axis=0)
    return (vv, recal, attn, sw)


# revision 23
# speedup vs baseline: 1.0086x; 1.0086x over previous
"""Trainium2 Bass kernel for BidirectionalCrossModalCausalAttention.

Shapes (hardcoded): B=64, S=4, C=1280, HID=256, H=W=32.
Sharding: data-parallel over batch: 8 samples per NeuronCore, weights replicated.

Per sample (x = visual_features[b] as (C, HW)):
  [k; v] = [Wk; Wv] @ x + [bk; bv]    -- one stacked fp32r GEMM, weights stationary
  scores = (q.k) / (max(|q|,eps) * max(|k|,eps));  attn = softmax over hw
  pooled = sum_hw v*attn;  visual_vector = pooled @ (Wp.T/1024) + bp
  gap = mean_hw x -> MLP -> softmax -> sensor_weights;  recal = sensor * sw
Softmax/attn/pooling run in half-batches of 4 samples so the first half
overlaps the second half's GEMMs.
"""
import numpy as np
from contextlib import ExitStack

import concourse.bass as bass
import concourse.tile as tile
from concourse import bacc, mybir
from concourse.bass_utils import run_bass_kernel_spmd

B, S, C, HID, H, W = 64, 4, 1280, 256, 32, 32
HW = H * W
NCORES = 8
BS = B // NCORES          # 8 samples per core
HB = BS // 2              # half-batch of 4
NK = C // 128             # 10 contraction tiles
NM = (2 * HID) // 128     # 4 output row-tiles (2 k + 2 v)
EPS = 1e-8
F32 = mybir.dt.float32
F32R = mybir.dt.float32r
AX = mybir.AxisListType.X
ALU = mybir.AluOpType
AF = mybir.ActivationFunctionType

_CACHE = {}


def _build():
    nc = bacc.Bacc("TRN2", target_bir_lowering=False, debug=False, num_devices=NCORES)

    dram = lambda nm, sh, kind: nc.dram_tensor(nm, sh, F32, kind=kind).ap()
    d_x = dram("x", [BS, C, HW], "ExternalInput")
    d_sensor = dram("sensor", [BS, S], "ExternalInput")
    d_sensorT = dram("sensorT", [S, BS], "ExternalInput")
    d_wallT = dram("wallT", [C, 2 * HID], "ExternalInput")
    d_wq = dram("wq", [S, HID], "ExternalInput")
    d_w1s = dram("w1s", [C, HID], "ExternalInput")
    d_w2 = dram("w2", [HID, S], "ExternalInput")
    d_wpTs = dram("wpTs", [HID, C], "ExternalInput")
    d_bk = dram("bk", [HID, 1], "ExternalInput")
    d_bv = dram("bv", [HID, 1], "ExternalInput")
    d_b1 = dram("b1", [HID, 1], "ExternalInput")
    d_bq_col = dram("bq_col", [HID, 1], "ExternalInput")
    d_bq_row = dram("bq_row", [1, HID], "ExternalInput")
    d_bp_row = dram("bp_row", [1, C], "ExternalInput")
    d_b2_row = dram("b2_row", [1, S], "ExternalInput")
    d_ones_col = dram("ones_col", [1, 128], "ExternalInput")
    d_ones_row = dram("ones_row", [1, BS], "ExternalInput")
    d_ones_ck = dram("ones_ck", [128, 1], "ExternalInput")
    d_vtmp = nc.dram_tensor("vtmp", [BS, 2, 128, HW], F32).ap()
    d_attn = dram("attn", [BS, HW], "ExternalOutput")
    d_vv = dram("vv", [BS, C], "ExternalOutput")
    d_sw = dram("sw", [BS, S], "ExternalOutput")
    d_recal = dram("recal", [BS, S], "ExternalOutput")

    with tile.TileContext(nc) as tc, ExitStack() as ctx:
        P = lambda **kw: ctx.enter_context(tc.tile_pool(**kw))
        wpool = P(name="w", bufs=1)
        xpool = P(name="x", bufs=14)
        kpool = P(name="k", bufs=2)
        vpool = P(name="v", bufs=4)
        spool = P(name="s", bufs=1)
        tpool = P(name="t", bufs=3)
        ps_main = P(name="pm", bufs=2, space="PSUM")    # 4 banks
        ps_small = P(name="psm", bufs=2, space="PSUM")  # 2 banks
        ps_bc = P(name="pbc", bufs=1, space="PSUM")     # 2 banks

        mm = nc.tensor.matmul
        act = nc.scalar.activation
        dve = nc.vector

        # ---- early weights (needed for main loop) ----
        wall = []
        for k in range(NK):
            t = wpool.tile([128, 2 * HID], F32R, name=f"wall_{k}")
            nc.sync.dma_start(out=t[:], in_=d_wallT[k * 128:(k + 1) * 128, :].bitcast(F32R))
            wall.append(t)
        wq_t = wpool.tile([S, HID], F32R, name="wq_t")
        nc.sync.dma_start(out=wq_t[:], in_=d_wq.bitcast(F32R))
        sensorT_t = wpool.tile([S, BS], F32R, name="sensorT_t")
        nc.sync.dma_start(out=sensorT_t[:], in_=d_sensorT.bitcast(F32R))
        sensor_t = wpool.tile([BS, S], F32, name="sensor_t")
        nc.sync.dma_start(out=sensor_t[:], in_=d_sensor)
        bq_row = wpool.tile([1, HID], F32R, name="bq_row")
        nc.sync.dma_start(out=bq_row[:], in_=d_bq_row.bitcast(F32R))
        bk_c, bv_c, b1_c, bq_c = [], [], [], []
        for j in range(2):
            for lst, src, nm in ((bk_c, d_bk, "bk"), (bv_c, d_bv, "bv"),
                                 (b1_c, d_b1, "b1"), (bq_c, d_bq_col, "bq")):
                t = wpool.tile([128, 1], F32, name=f"{nm}_c{j}")
                nc.sync.dma_start(out=t[:], in_=src[j * 128:(j + 1) * 128, :])
                lst.append(t)
        ones_col = wpool.tile([1, 128], F32R, name="ones_col")
        nc.sync.dma_start(out=ones_col[:], in_=d_ones_col.bitcast(F32R))
        ones_row = wpool.tile([1, BS], F32R, name="ones_row")
        nc.sync.dma_start(out=ones_row[:], in_=d_ones_row.bitcast(F32R))
        ones_ck = wpool.tile([128, 1], F32R, name="ones_ck")
        nc.sync.dma_start(out=ones_ck[:], in_=d_ones_ck.bitcast(F32R))

        # ---- q path: rq per half-batch + full qT ----
        rq_h, qT_r = [], []
        for h in range(2):
            q_ps = ps_small.tile([HB, HID], F32, tag="small", name=f"q_ps{h}")
            mm(out=q_ps[:], lhsT=sensorT_t[:, h * HB:(h + 1) * HB], rhs=wq_t[:],
               start=True, stop=False)
            mm(out=q_ps[:], lhsT=ones_row[:, 0:HB], rhs=bq_row[:], start=False, stop=True)
            q2s = tpool.tile([HB, HID], F32, tag="tq", name=f"q2s{h}")
            q2sum = spool.tile([HB, 1], F32, name=f"q2sum{h}")
            act(q2s[:], q_ps[:], AF.Square, accum_out=q2sum[:])
            qn = spool.tile([HB, 1], F32, name=f"qn{h}")
            act(qn[:], q2sum[:], AF.Sqrt)
            qnm = spool.tile([HB, 1], F32, name=f"qnm{h}")
            dve.tensor_scalar_max(qnm[:], qn[:], EPS)
            rq = spool.tile([HB, 1], F32, name=f"rq{h}")
            dve.reciprocal(rq[:], qnm[:])
            rq_h.append(rq)
        for j in range(2):
            p = ps_small.tile([128, BS], F32, tag="small", name=f"qT_ps{j}")
            mm(out=p[:], lhsT=wq_t[:, j * 128:(j + 1) * 128], rhs=sensorT_t[:],
               start=True, stop=True)
            t = spool.tile([128, BS], F32R, name=f"qT_r{j}")
            act(t[:], p[:], AF.Identity, bias=bq_c[j][:])
            qT_r.append(t)

        gapT = [spool.tile([128, BS], F32, name=f"gapT_{k}") for k in range(NK)]
        pooledT = [spool.tile([128, BS], F32, name=f"pooledT_{j}") for j in range(2)]
        qk_h = [spool.tile([HB, HW], F32, name=f"qk_h{h}") for h in range(2)]
        n2_h = [spool.tile([HB, HW], F32, name=f"n2_h{h}") for h in range(2)]

        attn_f_h = [None, None]

        def softmax_phase(h):
            """softmax + attn DMA for samples h*HB..h*HB+HB-1 (DVE/ACT only)."""
            knorm = tpool.tile([HB, HW], F32, tag="tb", name=f"knorm{h}")
            act(knorm[:], n2_h[h][:], AF.Sqrt)
            knm = tpool.tile([HB, HW], F32, tag="tb", name=f"knm{h}")
            dve.tensor_scalar_max(knm[:], knorm[:], EPS)
            rk = tpool.tile([HB, HW], F32, tag="tb", name=f"rk{h}")
            dve.reciprocal(rk[:], knm[:])
            scores = tpool.tile([HB, HW], F32, tag="tb", name=f"scores{h}")
            dve.tensor_tensor(scores[:], qk_h[h][:], rk[:], ALU.mult)
            maxs = spool.tile([HB, 1], F32, name=f"maxs{h}")
            dve.tensor_reduce(maxs[:], scores[:], AX, ALU.max)
            nrq = spool.tile([HB, 1], F32, name=f"nrq{h}")
            dve.tensor_scalar_mul(nrq[:], rq_h[h][:], -1.0)
            bias2 = spool.tile([HB, 1], F32, name=f"bias2{h}")
            dve.tensor_tensor(bias2[:], maxs[:], nrq[:], ALU.mult)
            e_t = tpool.tile([HB, HW], F32, tag="tb", name=f"e_t{h}")
            zsum = spool.tile([HB, 1], F32, name=f"zsum{h}")
            act(e_t[:], scores[:], AF.Exp, bias=bias2[:], scale=rq_h[h][:],
                accum_out=zsum[:])
            rz = spool.tile([HB, 1], F32, name=f"rz{h}")
            dve.reciprocal(rz[:], zsum[:])
            attn_f = tpool.tile([HB, HW], F32, tag="af", name=f"attn_f{h}", bufs=2)
            act(attn_f[:], e_t[:], AF.Copy, scale=rz[:])
            nc.sync.dma_start(out=d_attn[h * HB:(h + 1) * HB, :], in_=attn_f[:])
            attn_f_h[h] = attn_f

        def pooled_one(s):
            """attn-weighted v reduction for one sample."""
            h, i = s // HB, s % HB
            attn_f = attn_f_h[h]
            arow = tpool.tile([1, HW], F32R, tag="arow", name=f"arow_{s}", bufs=2)
            nc.sync.dma_start(out=arow[:], in_=attn_f[i:i + 1, :].bitcast(F32R))
            bc = ps_bc.tile([128, HW], F32, tag="bc", name=f"bc_{s}")
            for n in range(2):
                mm(out=bc[:, n * 512:(n + 1) * 512], lhsT=ones_col[:],
                   rhs=arow[0:1, n * 512:(n + 1) * 512], start=True, stop=True)
            for j in range(2):
                vt = vpool.tile([128, HW], F32, name=f"vld_{s}_{j}", tag="vsb")
                nc.gpsimd.dma_start(out=vt[:], in_=d_vtmp[s, j])
                scr = tpool.tile([128, HW], F32, tag="scr", name=f"scr_{s}_{j}", bufs=2)
                dve.tensor_tensor(scr[:], vt[:], bc[:], ALU.mult)
                dve.tensor_reduce(pooledT[j][:, s:s + 1], scr[:], AX, ALU.add)

        # ---- main per-sample loop ----
        for s in range(BS):
            h, i = s // HB, s % HB
            xt = [xpool.tile([128, HW], F32R, name=f"xt_{s}_{k}", tag="xt")
                  for k in range(NK)]
            for k in range(NK):
                eng = nc.gpsimd if k % 2 == 0 else nc.sync
                eng.dma_start(out=xt[k][:], in_=d_x[s, k * 128:(k + 1) * 128, :].bitcast(F32R))
            ks, k2s = [], []
            for m in range(NM):
                acc = ps_main.tile([128, HW], F32, tag="main", name=f"acc_{s}_{m}")
                for k in range(NK):
                    for n in range(2):
                        mm(out=acc[:, n * 512:(n + 1) * 512],
                           lhsT=wall[k][:, m * 128:(m + 1) * 128],
                           rhs=xt[k][:, n * 512:(n + 1) * 512],
                           start=(k == 0), stop=(k == NK - 1))
                if m < 2:
                    t1 = kpool.tile([128, HW], F32R, name=f"ksb_{s}_{m}", tag="ksb")
                    act(t1[:], acc[:], AF.Identity, bias=bk_c[m][:])
                    t2 = kpool.tile([128, HW], F32R, name=f"k2sb_{s}_{m}", tag="k2sb")
                    act(t2[:], acc[:], AF.Square, bias=bk_c[m][:])
                    ks.append(t1)
                    k2s.append(t2)
                else:
                    j = m - 2
                    t = vpool.tile([128, HW], F32, name=f"vsb_{s}_{j}", tag="vsb")
                    act(t[:], acc[:], AF.Identity, bias=bv_c[j][:])
                    nc.gpsimd.dma_start(out=d_vtmp[s, j], in_=t[:])
            for qi, (dst, rhs_t) in enumerate(((qk_h[h], ks), (n2_h[h], k2s))):
                row = tpool.tile([1, HW], F32, tag="row", name=f"row_{s}_{qi}", bufs=2)
                for n in range(2):
                    p = ps_small.tile([1, 512], F32, tag="small", name=f"rps_{s}_{n}_{qi}")
                    for j in range(2):
                        lhs = qT_r[j][:, s:s + 1] if qi == 0 else ones_ck[:]
                        mm(out=p[:], lhsT=lhs, rhs=rhs_t[j][:, n * 512:(n + 1) * 512],
                           start=(j == 0), stop=(j == 1))
                    act(row[0:1, n * 512:(n + 1) * 512], p[:], AF.Copy)
                nc.sync.dma_start(out=dst[i:i + 1, :], in_=row[:])
            for k in range(NK):
                dve.tensor_reduce(gapT[k][:, s:s + 1], xt[k][:].bitcast(F32), AX, ALU.add)
            if s == HB - 1:
                softmax_phase(0)
            if s >= HB:
                pooled_one(s - HB)
        softmax_phase(1)
        for s in range(HB, BS):
            pooled_one(s)

        # ---- late weights ----
        w1s, wpTs, w2t = [], [], []
        for k in range(NK):
            t = wpool.tile([128, HID], F32R, name=f"w1s_{k}")
            nc.sync.dma_start(out=t[:], in_=d_w1s[k * 128:(k + 1) * 128, :].bitcast(F32R))
            w1s.append(t)
        for j in range(2):
            t = wpool.tile([128, C], F32R, name=f"wpTs_{j}")
            nc.sync.dma_start(out=t[:], in_=d_wpTs[j * 128:(j + 1) * 128, :].bitcast(F32R))
            wpTs.append(t)
        for j in range(2):
            t = wpool.tile([128, S], F32R, name=f"w2t_{j}")
            nc.sync.dma_start(out=t[:], in_=d_w2[j * 128:(j + 1) * 128, :].bitcast(F32R))
            w2t.append(t)
        bp_row = wpool.tile([1, C], F32R, name="bp_row")
        nc.sync.dma_start(out=bp_row[:], in_=d_bp_row.bitcast(F32R))
        b2_row = wpool.tile([1, S], F32R, name="b2_row")
        nc.sync.dma_start(out=b2_row[:], in_=d_b2_row.bitcast(F32R))

        # ---- visual_vector = pooled @ (Wp.T/1024) + bp ----
        pooledT_r = []
        for j in range(2):
            t = spool.tile([128, BS], F32R, name=f"pooledT_r{j}")
            act(t[:], pooledT[j][:], AF.Copy)
            pooledT_r.append(t)
        vv_sb = spool.tile([BS, C], F32)
        for nchunk, n0 in ((512, 0), (512, 512), (256, 1024)):
            p = ps_small.tile([BS, 512], F32, tag="small", name=f"vv_ps_{n0}")
            for j in range(2):
                mm(out=p[:, :nchunk], lhsT=pooledT_r[j][:], rhs=wpTs[j][:, n0:n0 + nchunk],
                   start=(j == 0), stop=False)
            mm(out=p[:, :nchunk], lhsT=ones_row[:], rhs=bp_row[:, n0:n0 + nchunk],
               start=False, stop=True)
            act(vv_sb[:, n0:n0 + nchunk], p[:, :nchunk], AF.Copy)
        nc.sync.dma_start(out=d_vv[:, :], in_=vv_sb[:])

        # ---- gap MLP -> sensor weights ----
        gapT_r = []
        for k in range(NK):
            t = spool.tile([128, BS], F32R, name=f"gapT_r{k}")
            act(t[:], gapT[k][:], AF.Copy)
            gapT_r.append(t)
        hiddenT_r = []
        for j in range(2):
            p = ps_small.tile([128, BS], F32, tag="small", name=f"hid_ps{j}")
            for k in range(NK):
                mm(out=p[:], lhsT=w1s[k][:, j * 128:(j + 1) * 128], rhs=gapT_r[k][:],
                   start=(k == 0), stop=(k == NK - 1))
            t = spool.tile([128, BS], F32R, name=f"hiddenT_r{j}")
            act(t[:], p[:], AF.Relu, bias=b1_c[j][:])
            hiddenT_r.append(t)
        lg_ps = ps_small.tile([BS, S], F32, tag="small")
        for j in range(2):
            mm(out=lg_ps[:], lhsT=hiddenT_r[j][:], rhs=w2t[j][:], start=(j == 0), stop=False)
        mm(out=lg_ps[:], lhsT=ones_row[:], rhs=b2_row[:], start=False, stop=True)
        lmax = spool.tile([BS, 1], F32)
        dve.tensor_reduce(lmax[:], lg_ps[:], AX, ALU.max)
        nlmax = spool.tile([BS, 1], F32)
        dve.tensor_scalar_mul(nlmax[:], lmax[:], -1.0)
        le_t = spool.tile([BS, S], F32)
        lz = spool.tile([BS, 1], F32)
        act(le_t[:], lg_ps[:], AF.Exp, bias=nlmax[:], accum_out=lz[:])
        rlz = spool.tile([BS, 1], F32)
        dve.reciprocal(rlz[:], lz[:])
        sw_sb = spool.tile([BS, S], F32)
        act(sw_sb[:], le_t[:], AF.Copy, scale=rlz[:])
        nc.sync.dma_start(out=d_sw[:, :], in_=sw_sb[:])
        recal_sb = spool.tile([BS, S], F32)
        dve.tensor_tensor(recal_sb[:], sensor_t[:], sw_sb[:], ALU.mult)
        nc.sync.dma_start(out=d_recal[:, :], in_=recal_sb[:])

    nc.compile()
    return nc


def _prep_inputs(inputs):
    f = lambda a: np.ascontiguousarray(np.asarray(a, dtype=np.float32))
    sensor = f(inputs["sensor_features"])
    x = f(inputs["visual_features"]).reshape(B, C, HW)
    Wk, Wv = f(inputs["Wk"]), f(inputs["Wv"])
    wallT = np.ascontiguousarray(np.concatenate([Wk.T, Wv.T], axis=1))  # (C, 2*HID)
    shared = {
        "wallT": wallT,
        "wq": f(inputs["Wq"]),
        "w1s": np.ascontiguousarray(f(inputs["W1"]) / HW),
        "w2": f(inputs["W2"]),
        "wpTs": np.ascontiguousarray(f(inputs["Wp"]).T / HW),
        "bk": f(inputs["bk"]).reshape(HID, 1),
        "bv": f(inputs["bv"]).reshape(HID, 1),
        "b1": f(inputs["b1"]).reshape(HID, 1),
        "bq_col": f(inputs["bq"]).reshape(HID, 1),
        "bq_row": f(inputs["bq"]).reshape(1, HID),
        "bp_row": f(inputs["bp"]).reshape(1, C),
        "b2_row": f(inputs["b2"]).reshape(1, S),
        "ones_col": np.ones((1, 128), np.float32),
        "ones_row": np.ones((1, BS), np.float32),
        "ones_ck": np.ones((128, 1), np.float32),
    }
    in_maps = []
    for i in range(NCORES):
        sl = slice(i * BS, (i + 1) * BS)
        m = dict(shared)
        m["x"] = np.ascontiguousarray(x[sl])
        m["sensor"] = np.ascontiguousarray(sensor[sl])
        m["sensorT"] = np.ascontiguousarray(sensor[sl].T)
        in_maps.append(m)
    return in_maps


def kernel(**inputs):
    if "nc" not in _CACHE:
        _CACHE["nc"] = _build()
    nc = _CACHE["nc"]
    in_maps = _prep_inputs(inputs)
    res = run_bass_kernel_spmd(nc, in_maps, list(range(NCORES))).results
    vv = np.concatenate([r["vv"] for r in res], axis=0)
    recal = np.concatenate([r["recal"] for r in res], axis=0)
    attn = np.concatenate([r["attn"] for r in res], axis=0).reshape(B, 1, H, W)
    sw = np.concatenate([r["sw"] for r in res], axis=0)
    return (vv, recal, attn, sw)
